# revision 30
# baseline (speedup 1.0000x reference)
"""MinimalGRU (2-layer) Trainium2 Bass kernel, data-parallel over batch on 8 cores.

Full inputs in, full output out. Per core: 4 sequences.

Per step the hidden state h lives in two layouts: scattered (partition 32*j+b
holds h[b, 256*j:256*(j+1)], for elementwise ops) and transposed ht [128, 32]
(column 4*k+b holds h[b, 128*k:128*(k+1)], the stationary operand of the gate
matmuls). Gates are computed in two phases (o-gates then u-gates) into two
separate PSUM banks. Pre-activations (input projection + biases, precomputed
as a parallel GEMM) are added by DVE tensor_tensor ops on scattered-layout
windows: the o-half add and relu/sub hide under the u-phase matmuls, leaving
only add -> sigmoid -> mult -> add as the post-matmul tail. The two layers
run LAG steps apart and their PE phases are emitted in a rotation

  [L0 gates(s)] [L1 tp(s'-1), GEMM filler, L1 ht copy] [L1 gates(s')]
  [L0 tp(s), GEMM filler, L0 ht copy] ...

so each layer's elementwise tail executes while the other layer's gate
matmuls keep the PE busy, and layer-1's input GEMM is cut into 64 N=256
matmuls spread one per rebuild slot as PE filler over the ht-copy latency.
Layer-0 outputs never touch DRAM except as that GEMM's feed every P1_WIN
steps.
"""

import os
import contextlib
import numpy as np
import ml_dtypes

import concourse.bass as bass  # noqa: F401
import concourse.mybir as mybir
from concourse import bacc
from concourse.tile import TileContext
from concourse.bass_utils import run_bass_kernel_spmd

BF16 = ml_dtypes.bfloat16
F32 = np.float32

H = 1024
DX = 512
G = 2 * H          # 2048 gate columns (u block then o block, natural order)
B = 32
NCORES = 8
BL = B // NCORES   # 4 sequences per core
T = int(os.environ.get("GRU_T", "512"))

UPRE_WIN = 4       # pre-activation (scattered layout) DMA window (steps)
P1_WIN = 32        # tokens gathered per layer-1 pre GEMM (= steps)
OUT_WIN = 8        # L1 output DMA window (steps)
LAG = 68           # layer-1 lag: one extra P1 window, since the layer-1
                   # input GEMM is spread one matmul per step as PE filler

_CACHE: dict = {}


class _LS:
    pass


def _make_layer(nc, tc, stack, layer, w_t, pre_d, h0t_t, h0s_t, idt_t,
                wih1_t=None, b1f_t=None, pre1_d=None, out_dram=None):
    fp32 = mybir.dt.float32
    bf16 = mybir.dt.bfloat16
    L = _LS()
    L.layer = layer
    L.w_t = w_t
    L.pre_d = pre_d
    L.idt_t = idt_t
    L.wih1_t = wih1_t
    L.b1f_t = b1f_t
    L.pre1_d = pre1_d
    L.out_dram = out_dram
    L.pbase = 0 if layer == 0 else 32   # partition base of pre tiles / I4 rows
    ctx = stack.enter_context
    L.prew_pool = ctx(tc.tile_pool(name=f"prew{layer}", bufs=2))
    L.stage_pool = ctx(tc.tile_pool(name=f"stage{layer}",
                                    bufs=3 if layer == 0 else 2))
    L.tmp_pool = ctx(tc.tile_pool(name=f"tmp{layer}", bufs=4))
    L.gpu_pool = ctx(tc.tile_pool(name=f"gpu{layer}", bufs=1, space="PSUM"))
    L.gpo_pool = ctx(tc.tile_pool(name=f"gpo{layer}", bufs=1, space="PSUM"))
    L.tps_pool = ctx(tc.tile_pool(name=f"tps{layer}", bufs=1, space="PSUM"))
    if layer == 0:
        L.o0t_pool = ctx(tc.tile_pool(name="o0t0", bufs=2))
        L.pps_pool = ctx(tc.tile_pool(name="pps0", bufs=2, space="PSUM"))
        L.psb_pool = ctx(tc.tile_pool(name="psb0", bufs=2))
    else:
        L.ht_pool = ctx(tc.tile_pool(name="ht1", bufs=3))
    L.gpu = L.gpu_pool.tile([128, 512], fp32, tag="gpu", name=f"gpuf{layer}")
    L.gpo = L.gpo_pool.tile([128, 512], fp32, tag="gpo", name=f"gpof{layer}")
    L.tp = L.tps_pool.tile([128, 512], fp32, tag="tp", name=f"tpf{layer}")
    nc.vector.memset(L.gpu[:], 0.0)
    nc.vector.memset(L.gpo[:], 0.0)
    L.uprew_pool = ctx(tc.tile_pool(name=f"uprew{layer}", bufs=2))
    L.uprew_tiles = [
        L.uprew_pool.tile([128, UPRE_WIN, 256], bf16, tag="uprew",
                          name=f"uprewf{layer}_{i}") for i in range(2)]
    L.oprew_pool = ctx(tc.tile_pool(name=f"oprew{layer}", bufs=2))
    L.oprew_tiles = [
        L.oprew_pool.tile([128, UPRE_WIN, 256], bf16, tag="oprew",
                          name=f"oprewf{layer}_{i}") for i in range(2)]
    for t_ in L.uprew_tiles + L.oprew_tiles:
        nc.vector.memset(t_[:], 0.0)   # unused partitions must stay finite
    L.gemm_src = None       # previous-window o0t being GEMMed (layer 0)
    L.gemm_w = -1
    L.gemm_i = 0
    L.gemm_pp = None
    L.gemm_psb = None
    L.prev_h = h0s_t[:, :]
    L.ht_src = h0t_t
    L.prew_cur = None
    L.stage_cur = None
    L.o0t_cur = None
    return L


def _emit_sprew_dma(nc, L, ts, half):
    """Prefetch a scattered-layout pre window (row 32j+b gets
    pre[b, t, off+256j : off+256j+256]) starting at step ts.
    half 0 = u (cols 0:H), half 1 = o (cols H:G)."""
    if ts >= T:
        return
    tiles = L.uprew_tiles if half == 0 else L.oprew_tiles
    off = 0 if half == 0 else H
    cur = tiles[(ts // UPRE_WIN) % 2]
    for j in range(4):
        dst = cur[32 * j:32 * j + 4, :, :]
        c0 = off + 256 * j
        if L.layer == 0:
            src = L.pre_d[:, ts:ts + UPRE_WIN, c0:c0 + 256]
        else:
            w = ts // P1_WIN
            s0 = ts % P1_WIN
            src = (L.pre_d[w][:, c0:c0 + 256]
                   .rearrange("(s b) c -> b s c", b=4)
                   [:, s0:s0 + UPRE_WIN, :])
        nc.sync.dma_start(dst, src)


def _emit_gates(nc, L, ts):
    """PE: u-phase then o-phase gate matmuls; ACT/DVE: elementwise to hn."""
    fp32 = mybir.dt.float32
    layer = L.layer

    if layer == 1:
        s_st = ts % OUT_WIN
        if s_st == 0:
            L.stage_cur = L.stage_pool.tile([128, OUT_WIN, 256], fp32,
                                            tag="stage", name="stagew")
        hn = L.stage_cur[:, s_st, :]
    else:
        L.stage_cur = L.stage_pool.tile([128, 256], fp32, tag="stage",
                                        name="stage")
        hn = L.stage_cur[:, :]
        sw = ts % P1_WIN
        if sw == 0:
            L.o0t_cur = L.o0t_pool.tile([128, 8, 4 * P1_WIN],
                                        mybir.dt.bfloat16,
                                        tag="o0t", name="o0t")

    if ts % UPRE_WIN == 0:          # prefetch the next pre windows
        _emit_sprew_dma(nc, L, ts + UPRE_WIN, 0)
        _emit_sprew_dma(nc, L, ts + UPRE_WIN, 1)

    ht = L.ht_src
    sw2 = ts % UPRE_WIN
    # ---- o-phase first: gpo[32j+b, q] = h[b, :] @ Wo[:, 256j+q]
    for k in range(8):
        lhsT = (ht[:, k, :] if ht.ndim == 3 else ht[:, 4 * k:4 * k + 4])
        for j in range(4):
            nc.tensor.matmul(
                L.gpo[32 * j:32 * j + 4, 0:256],
                lhsT,
                L.w_t[k][:, H + 256 * j:H + 256 * j + 256],
                start=(k == 0), stop=(k == 7),
                tile_position=(0, 32 * j),
                skip_group_check=True,
            )
    # o pre add + d = relu(go) - h (DVE), hidden under the u-phase
    ocur = L.oprew_tiles[(ts // UPRE_WIN) % 2]
    nc.vector.tensor_tensor(L.gpo[0:100, 0:256], L.gpo[0:100, 0:256],
                            ocur[0:100, sw2, :], mybir.AluOpType.add)
    d = L.tmp_pool.tile([128, 256], fp32, tag="d", name="d")
    nc.vector.scalar_tensor_tensor(
        d[0:100, :], L.gpo[0:100, 0:256], 0.0, L.prev_h[0:100, :],
        mybir.AluOpType.max, mybir.AluOpType.subtract)
    # ---- u-phase
    for k in range(8):
        lhsT = (ht[:, k, :] if ht.ndim == 3 else ht[:, 4 * k:4 * k + 4])
        for j in range(4):
            nc.tensor.matmul(
                L.gpu[32 * j:32 * j + 4, 0:256],
                lhsT,
                L.w_t[k][:, 256 * j:256 * j + 256],
                start=(k == 0), stop=(k == 7),
                tile_position=(0, 32 * j),
                skip_group_check=True,
            )
    # ---- tail: u' = sigmoid(-(gu+pu));  t = u'*d;  hn = h + t
    ucur = L.uprew_tiles[(ts // UPRE_WIN) % 2]
    nc.vector.tensor_tensor(L.gpu[0:100, 0:256], L.gpu[0:100, 0:256],
                            ucur[0:100, sw2, :], mybir.AluOpType.add)
    up = L.tmp_pool.tile([128, 256], fp32, tag="up", name="up")
    nc.scalar.activation(up[0:100, :], L.gpu[0:100, 0:256],
                         mybir.ActivationFunctionType.Sigmoid, scale=-1.0)
    nc.vector.tensor_tensor(d[0:100, :], d[0:100, :], up[0:100, :],
                            mybir.AluOpType.mult)
    nc.vector.tensor_tensor(hn[0:100, :], L.prev_h[0:100, :],
                            d[0:100, :], mybir.AluOpType.add)
    L.hn = hn
    L.prev_h = hn


def _emit_rebuild(nc, L, ts):
    """PE: transpose hn into tp; then I4 for step ts+1; DVE: copy tp -> ht."""
    bf16 = mybir.dt.bfloat16
    layer = L.layer
    hn = L.hn
    tp = L.tp

    for k in range(8):
        j, kk = k // 2, k % 2
        nc.tensor.matmul(
            tp[:, 4 * k:4 * k + 4],
            hn[32 * j:32 * j + 4, 128 * kk:128 * kk + 128],
            L.id_tr[32 * j:32 * j + 4, 0:4],
            tile_position=(32 * j, 0),
            skip_group_check=True,
        )
    _emit_gemm_step(nc, L.gemmL)  # PE filler while the copy below lands
    # the ht copies run on the near-idle ACT engine: on DVE they queue
    # behind the other layer's elementwise tail (~650ns extra PE wait)
    if layer == 0:
        sw = ts % P1_WIN
        nc.scalar.activation(
            L.o0t_cur[:, :, 4 * sw:4 * sw + 4],
            tp[:, 0:32].rearrange("p (k b) -> p k b", b=4),
            mybir.ActivationFunctionType.Copy)
        L.ht_src = L.o0t_cur[:, :, 4 * sw:4 * sw + 4]
        if sw == P1_WIN - 1:
            # hand this window's o0t to the spread GEMM (one matmul per
            # rebuild slot over the following window's steps)
            L.gemm_src = L.o0t_cur
            L.gemm_w = ts // P1_WIN
            L.gemm_i = 0
    else:
        ht = L.ht_pool.tile([128, 32], bf16, tag="ht", name="ht")
        nc.scalar.activation(ht[:, :], tp[:, 0:32],
                             mybir.ActivationFunctionType.Copy)
        L.ht_src = ht
        s_st = ts % OUT_WIN
        if s_st == OUT_WIN - 1:
            w0 = ts - (OUT_WIN - 1)
            for j in range(4):
                nc.sync.dma_start(
                    L.out_dram[:, w0:w0 + OUT_WIN, 256 * j:256 * j + 256],
                    L.stage_cur[32 * j:32 * j + 4, :, :],
                )


def _emit_gemm_step(nc, L):
    """Emit one matmul of layer-0's pending layer-1 input GEMM.

    The GEMM is cut into 64 N=256 matmuls so each of the two per-step
    rebuild slots gets one as PE filler over the ht-copy latency."""
    if L is None or L.gemm_src is None:
        return
    fp32 = mybir.dt.float32
    bf16 = mybir.dt.bfloat16
    i = L.gemm_i
    n, k = i // 8, i % 8      # n: 8 column slices of 256, k: 8 K chunks
    if k == 0:
        # full-bank tile (half used) so alternating pp tiles never share a
        # bank: PE-write(pp_n) + DVE-read(pp_{n-1}) same-bank is fatal
        L.gemm_pp = L.pps_pool.tile([128, 512], fp32, tag="pps", name="pps")
        if n == 0:
            L.gemm_psb = L.psb_pool.tile([128, G], bf16, tag="psb",
                                         name="psb")
    nc.tensor.matmul(
        L.gemm_pp[:, 0:256],
        L.gemm_src[:, k, :],
        L.wih1_t[k][:, 256 * n:256 * n + 256],
        start=(k == 0), stop=(k == 7),
        skip_group_check=True,
    )
    if k == 7:
        nc.vector.tensor_tensor(
            L.gemm_psb[:, 256 * n:256 * n + 256], L.gemm_pp[:, 0:256],
            L.b1f_t[:, 256 * n:256 * n + 256], mybir.AluOpType.add)
    L.gemm_i += 1
    if L.gemm_i == 64:
        nc.sync.dma_start(L.pre1_d[L.gemm_w, :, :], L.gemm_psb[:, :])
        L.gemm_src = None


def _build():
    fp32 = mybir.dt.float32
    bf16 = mybir.dt.bfloat16
    nc = bacc.Bacc("TRN2", target_bir_lowering=False, debug=False,
                   num_devices=NCORES)

    xt = nc.dram_tensor("xt", [DX, BL * T], bf16, kind="ExternalInput")
    w0p = nc.dram_tensor("w0p", [H, G], bf16, kind="ExternalInput")
    w1p = nc.dram_tensor("w1p", [H, G], bf16, kind="ExternalInput")
    wih0p = nc.dram_tensor("wih0p", [DX, G], bf16, kind="ExternalInput")
    wih1p = nc.dram_tensor("wih1p", [H, G], bf16, kind="ExternalInput")
    b0f = nc.dram_tensor("b0f", [128, G], fp32, kind="ExternalInput")
    b1f = nc.dram_tensor("b1f", [128, G], fp32, kind="ExternalInput")
    h0t = nc.dram_tensor("h0t", [128, 32], bf16, kind="ExternalInput")
    h1t = nc.dram_tensor("h1t", [128, 32], bf16, kind="ExternalInput")
    h0s = nc.dram_tensor("h0s", [128, 256], fp32, kind="ExternalInput")
    h1s = nc.dram_tensor("h1s", [128, 256], fp32, kind="ExternalInput")
    idt = nc.dram_tensor("idt", [128, 4], fp32, kind="ExternalInput")
    out = nc.dram_tensor("out", [BL, T, H], fp32, kind="ExternalOutput")

    pre0_d = nc.dram_tensor("pre0_d", [BL, T, G], bf16, kind="Internal")
    pre1_d = nc.dram_tensor("pre1_d", [T // P1_WIN, 128, G], bf16,
                            kind="Internal")

    with TileContext(nc) as tc:
        with tc.tile_pool(name="wconst", bufs=1) as wconst:
            w0_t = [wconst.tile([128, G], bf16, tag=f"w0_{k}", name=f"w0_{k}")
                    for k in range(8)]
            w1_t = [wconst.tile([128, G], bf16, tag=f"w1_{k}", name=f"w1_{k}")
                    for k in range(8)]
            wih1_t = [wconst.tile([128, G], bf16, tag=f"wih1_{k}",
                                  name=f"wih1_{k}") for k in range(8)]
            b1f_t = wconst.tile([128, G], fp32, tag="b1f", name="b1f")
            h0t_t = wconst.tile([128, 32], bf16, tag="h0t", name="h0t")
            h1t_t = wconst.tile([128, 32], bf16, tag="h1t", name="h1t")
            h0s_t = wconst.tile([128, 256], fp32, tag="h0s", name="h0s")
            h1s_t = wconst.tile([128, 256], fp32, tag="h1s", name="h1s")
            idt_t = wconst.tile([128, 4], fp32, tag="idt", name="idt")

            # ---- P1: layer-0 input GEMM -> pre0_d (bf16, natural order)
            with (
                tc.tile_pool(name="p1x", bufs=1) as p1x,
                tc.tile_pool(name="p1ps", bufs=2, space="PSUM") as p1ps,
                tc.tile_pool(name="p1o", bufs=2) as p1o,
            ):
                b0f_t = p1x.tile([128, G], fp32, tag="b0f", name="b0f")
                xt_t = [p1x.tile([128, BL * T], bf16, tag=f"xt{k}",
                                 name=f"xtt{k}") for k in range(4)]
                wih0_t = [p1x.tile([128, G], bf16, tag=f"wih0_{k}",
                                   name=f"wih0_{k}") for k in range(4)]
                # P1's own inputs first: the first GEMM matmul only waits on
                # these; the recurrence weights queue behind on SP
                for k in range(4):
                    nc.sync.dma_start(xt_t[k][:, :],
                                      xt[128 * k:128 * k + 128, :])
                    nc.sync.dma_start(wih0_t[k][:, :],
                                      wih0p[128 * k:128 * k + 128, :])
                nc.sync.dma_start(b0f_t[:, :], b0f[:, :])
                for k in range(8):
                    nc.sync.dma_start(w0_t[k][:, :],
                                      w0p[128 * k:128 * k + 128, :])
                    nc.sync.dma_start(w1_t[k][:, :],
                                      w1p[128 * k:128 * k + 128, :])
                    nc.sync.dma_start(wih1_t[k][:, :],
                                      wih1p[128 * k:128 * k + 128, :])
                nc.sync.dma_start(b1f_t[:, :], b1f[:, :])
                for dst, src in ((h0t_t, h0t), (h1t_t, h1t), (h0s_t, h0s),
                                 (h1s_t, h1s), (idt_t, idt)):
                    nc.sync.dma_start(dst[:, :], src[:, :])
                for m in range(BL * T // 128):
                    po = p1o.tile([128, G], bf16, tag="po", name="po")
                    for n in range(4):
                        pp = p1ps.tile([128, 512], fp32, tag="pp", name="pp")
                        for k in range(4):
                            nc.tensor.matmul(
                                pp[:, :],
                                xt_t[k][:, 128 * m:128 * m + 128],
                                wih0_t[k][:, 512 * n:512 * n + 512],
                                start=(k == 0), stop=(k == 3),
                            )
                        nc.vector.tensor_tensor(
                            po[:, 512 * n:512 * n + 512], pp[:, :],
                            b0f_t[:, 512 * n:512 * n + 512],
                            mybir.AluOpType.add)
                    bb = m // (T // 128)
                    t0 = 128 * (m % (T // 128))
                    nc.sync.dma_start(pre0_d[bb, t0:t0 + 128, :], po[:, :])

            tc.strict_bb_all_engine_barrier()
            with contextlib.ExitStack() as stack:
                L0 = _make_layer(nc, tc, stack, 0, w0_t, pre0_d, h0t_t,
                                 h0s_t, idt_t, wih1_t=wih1_t, b1f_t=b1f_t,
                                 pre1_d=pre1_d)
                L1 = _make_layer(nc, tc, stack, 1, w1_t, pre1_d, h1t_t,
                                 h1s_t, idt_t, out_dram=out)
                for L in (L0, L1):
                    L.id_tr = idt_t     # fp32 identity for the transposes
                    L.gemmL = L0        # both rebuild slots drain L0's GEMM
                _emit_sprew_dma(nc, L0, 0, 0)
                _emit_sprew_dma(nc, L0, 0, 1)
                for tt in range(T + LAG + 1):
                    if tt < T:
                        _emit_gates(nc, L0, tt)
                    if tt == LAG - 2:
                        # L1's first pre windows: emitted only after L0's
                        # window-0 GEMM wrote pre1_d[0] (program order).
                        _emit_sprew_dma(nc, L1, 0, 0)
                        _emit_sprew_dma(nc, L1, 0, 1)
                    s1 = tt - LAG - 1
                    if 0 <= s1 < T:
                        _emit_rebuild(nc, L1, s1)
                    s1b = tt - LAG
                    if 0 <= s1b < T:
                        _emit_gates(nc, L1, s1b)
                    if tt < T:
                        _emit_rebuild(nc, L0, tt)
                        if tt <= LAG:
                            # L1's rebuild slot doesn't exist yet: pull an
                            # extra GEMM matmul so window 0 drains in time
                            _emit_gemm_step(nc, L0)
                    else:
                        # drain the last window's spread GEMM
                        _emit_gemm_step(nc, L0)
                        _emit_gemm_step(nc, L0)

    nc.compile()
    return nc


def _prep_core(inputs, c, shared):
    x = inputs["x"][BL * c: BL * c + BL, :T]          # [4, T, DX]
    xt = np.ascontiguousarray(
        x.transpose(2, 0, 1).reshape(DX, BL * T)).astype(BF16)

    def hscat(hv):                                    # [4, H] -> [128, 256]
        o = np.zeros((128, 256), F32)
        for j in range(4):
            o[32 * j: 32 * j + 4, :] = hv[:, 256 * j: 256 * j + 256]
        return o

    def htr(hv):                                      # [4, H] -> [128, 32]
        o = np.zeros((128, 32), F32)
        for k in range(8):
            o[:, 4 * k: 4 * k + 4] = hv[:, 128 * k: 128 * k + 128].T
        return o

    h0 = inputs["hx"][0, BL * c: BL * c + BL]
    h1 = inputs["hx"][1, BL * c: BL * c + BL]
    return {
        "xt": xt,
        "h0t": htr(h0).astype(BF16), "h1t": htr(h1).astype(BF16),
        "h0s": hscat(h0), "h1s": hscat(h1),
        **shared,
    }


def get_nc():
    nc = _CACHE.get("nc")
    if nc is None:
        nc = _build()
        _CACHE["nc"] = nc
    return nc


def make_in_maps(inputs):
    inputs = {k: np.asarray(v) for k, v in inputs.items()}
    idt = np.zeros((128, 4), F32)
    for j in range(4):
        for b in range(4):
            idt[32 * j + b, b] = 1.0
    shared = {
        "w0p": np.ascontiguousarray(inputs["w_hh_l0"].T).astype(BF16),
        "w1p": np.ascontiguousarray(inputs["w_hh_l1"].T).astype(BF16),
        "wih0p": np.ascontiguousarray(inputs["w_ih_l0"].T).astype(BF16),
        "wih1p": np.ascontiguousarray(inputs["w_ih_l1"].T).astype(BF16),
        "b0f": np.broadcast_to(
            (inputs["b_ih_l0"] + inputs["b_hh_l0"]),
            (128, G)).astype(F32).copy(),
        "b1f": np.broadcast_to(
            (inputs["b_ih_l1"] + inputs["b_hh_l1"]),
            (128, G)).astype(F32).copy(),
        "idt": idt,
    }
    return [_prep_core(inputs, c, shared) for c in range(NCORES)]


def kernel(**inputs) -> np.ndarray:
    nc = get_nc()
    in_maps = make_in_maps(inputs)
    try:
        res = run_bass_kernel_spmd(nc, in_maps, core_ids=list(range(NCORES)))
    except Exception:
        # a previously wedged device often recovers on the next attempt
        import time
        time.sleep(2.0)
        res = run_bass_kernel_spmd(nc, in_maps, core_ids=list(range(NCORES)))
    out = np.concatenate([res.results[c]["out"] for c in range(NCORES)],
                         axis=0)
    return np.asarray(out, np.float32)


# revision 35
# speedup vs baseline: 3.2491x; 3.2491x over previous
"""MinimalGRU (2-layer) Trainium2 Bass kernel, data-parallel over batch on 8 cores.

Full inputs in, full output out. Per core: 4 sequences.

Per step the hidden state h lives in two layouts: scattered (partition 32*j+b
holds h[b, 256*j:256*(j+1)], for elementwise ops) and transposed ht [128, 32]
(column 4*k+b holds h[b, 128*k:128*(k+1)], the stationary operand of the gate
matmuls). Gates are computed in two phases (o-gates then u-gates) into two
separate PSUM banks. Pre-activations (input projection + biases, precomputed
as a parallel GEMM) are added by DVE tensor_tensor ops on scattered-layout
windows: the o-half add and relu/sub hide under the u-phase matmuls, leaving
only add -> sigmoid -> mult -> add as the post-matmul tail. The two layers
run LAG steps apart and their PE phases are emitted in a rotation

  [L0 gates(s)] [L1 tp(s'-1), GEMM filler, L1 ht copy] [L1 gates(s')]
  [L0 tp(s), GEMM filler, L0 ht copy] ...

so each layer's elementwise tail executes while the other layer's gate
matmuls keep the PE busy, and layer-1's input GEMM is cut into 64 N=256
matmuls spread one per rebuild slot as PE filler over the ht-copy latency.
Layer-0 outputs never touch DRAM except as that GEMM's feed every P1_WIN
steps.
"""

import os
import contextlib
import numpy as np
import ml_dtypes

import concourse.bass as bass  # noqa: F401
import concourse.mybir as mybir
from concourse import bacc
from concourse.tile import TileContext
from concourse.bass_utils import run_bass_kernel_spmd

BF16 = ml_dtypes.bfloat16
F32 = np.float32

H = 1024
DX = 512
G = 2 * H          # 2048 gate columns (u block then o block, natural order)
B = 32
NCORES = 8
BL = B // NCORES   # 4 sequences per core
T = int(os.environ.get("GRU_T", "512"))

UPRE_WIN = 4       # pre-activation (scattered layout) DMA window (steps)
P1_WIN = 32        # tokens gathered per layer-1 pre GEMM (= steps)
OUT_WIN = 8        # L1 output DMA window (steps)
LAG = 68           # layer-1 lag: one extra P1 window, since the layer-1
                   # input GEMM is spread one matmul per step as PE filler

_CACHE: dict = {}


class _LS:
    pass


def _make_layer(nc, tc, stack, layer, w_t, pre_d, h0t_t, h0s_t, idt_t,
                wih1_t=None, b1f_t=None, pre1_d=None, out_dram=None):
    fp32 = mybir.dt.float32
    bf16 = mybir.dt.bfloat16
    L = _LS()
    L.layer = layer
    L.w_t = w_t
    L.pre_d = pre_d
    L.idt_t = idt_t
    L.wih1_t = wih1_t
    L.b1f_t = b1f_t
    L.pre1_d = pre1_d
    L.out_dram = out_dram
    L.pbase = 0 if layer == 0 else 32   # partition base of pre tiles / I4 rows
    ctx = stack.enter_context
    L.prew_pool = ctx(tc.tile_pool(name=f"prew{layer}", bufs=2))
    L.stage_pool = ctx(tc.tile_pool(name=f"stage{layer}",
                                    bufs=3 if layer == 0 else 2))
    L.tmp_pool = ctx(tc.tile_pool(name=f"tmp{layer}", bufs=4))
    L.gpu_pool = ctx(tc.tile_pool(name=f"gpu{layer}", bufs=1, space="PSUM"))
    L.gpo_pool = ctx(tc.tile_pool(name=f"gpo{layer}", bufs=1, space="PSUM"))
    L.tps_pool = ctx(tc.tile_pool(name=f"tps{layer}", bufs=1, space="PSUM"))
    if layer == 0:
        L.o0t_pool = ctx(tc.tile_pool(name="o0t0", bufs=2))
        L.pps_pool = ctx(tc.tile_pool(name="pps0", bufs=2, space="PSUM"))
        L.psb_pool = ctx(tc.tile_pool(name="psb0", bufs=2))
    else:
        L.ht_pool = ctx(tc.tile_pool(name="ht1", bufs=3))
    L.gpu = L.gpu_pool.tile([128, 512], fp32, tag="gpu", name=f"gpuf{layer}")
    L.gpo = L.gpo_pool.tile([128, 512], fp32, tag="gpo", name=f"gpof{layer}")
    L.tp = L.tps_pool.tile([128, 512], fp32, tag="tp", name=f"tpf{layer}")
    nc.vector.memset(L.gpu[:], 0.0)
    nc.vector.memset(L.gpo[:], 0.0)
    L.uprew_pool = ctx(tc.tile_pool(name=f"uprew{layer}", bufs=2))
    L.uprew_tiles = [
        L.uprew_pool.tile([128, UPRE_WIN, 256], bf16, tag="uprew",
                          name=f"uprewf{layer}_{i}") for i in range(2)]
    L.oprew_pool = ctx(tc.tile_pool(name=f"oprew{layer}", bufs=2))
    L.oprew_tiles = [
        L.oprew_pool.tile([128, UPRE_WIN, 256], bf16, tag="oprew",
                          name=f"oprewf{layer}_{i}") for i in range(2)]
    for t_ in L.uprew_tiles + L.oprew_tiles:
        nc.vector.memset(t_[:], 0.0)   # unused partitions must stay finite
    L.gemm_src = None       # previous-window o0t being GEMMed (layer 0)
    L.gemm_w = -1
    L.gemm_i = 0
    L.gemm_pp = None
    L.gemm_psb = None
    L.prev_h = h0s_t[:, :]
    L.ht_src = h0t_t
    L.prew_cur = None
    L.stage_cur = None
    L.o0t_cur = None
    return L


def _emit_sprew_dma(nc, L, ts, half):
    """Prefetch a scattered-layout pre window (row 32j+b gets
    pre[b, t, off+256j : off+256j+256]) starting at step ts.
    half 0 = u (cols 0:H), half 1 = o (cols H:G)."""
    if ts >= T:
        return
    tiles = L.uprew_tiles if half == 0 else L.oprew_tiles
    off = 0 if half == 0 else H
    cur = tiles[(ts // UPRE_WIN) % 2]
    for j in range(4):
        dst = cur[32 * j:32 * j + 4, :, :]
        c0 = off + 256 * j
        if L.layer == 0:
            src = L.pre_d[:, ts:ts + UPRE_WIN, c0:c0 + 256]
        else:
            w = ts // P1_WIN
            s0 = ts % P1_WIN
            src = (L.pre_d[w][:, c0:c0 + 256]
                   .rearrange("(s b) c -> b s c", b=4)
                   [:, s0:s0 + UPRE_WIN, :])
        nc.sync.dma_start(dst, src)


def _emit_gates(nc, L, ts):
    """PE: u-phase then o-phase gate matmuls; ACT/DVE: elementwise to hn."""
    fp32 = mybir.dt.float32
    layer = L.layer

    if layer == 1:
        s_st = ts % OUT_WIN
        if s_st == 0:
            L.stage_cur = L.stage_pool.tile([128, OUT_WIN, 256], fp32,
                                            tag="stage", name="stagew")
        hn = L.stage_cur[:, s_st, :]
    else:
        L.stage_cur = L.stage_pool.tile([128, 256], fp32, tag="stage",
                                        name="stage")
        hn = L.stage_cur[:, :]
        sw = ts % P1_WIN
        if sw == 0:
            L.o0t_cur = L.o0t_pool.tile([128, 8, 4 * P1_WIN],
                                        mybir.dt.bfloat16,
                                        tag="o0t", name="o0t")

    if ts % UPRE_WIN == 0:          # prefetch the next pre windows
        _emit_sprew_dma(nc, L, ts + UPRE_WIN, 0)
        _emit_sprew_dma(nc, L, ts + UPRE_WIN, 1)

    ht = L.ht_src
    sw2 = ts % UPRE_WIN
    # ---- o-phase first: gpo[32j+b, q] = h[b, :] @ Wo[:, 256j+q]
    for k in range(8):
        lhsT = (ht[:, k, :] if ht.ndim == 3 else ht[:, 4 * k:4 * k + 4])
        for j in range(4):
            nc.tensor.matmul(
                L.gpo[32 * j:32 * j + 4, 0:256],
                lhsT,
                L.w_t[k][:, H + 256 * j:H + 256 * j + 256],
                start=(k == 0), stop=(k == 7),
                tile_position=(0, 32 * j),
                skip_group_check=True,
            )
    # o pre add + d = relu(go) - h (DVE), hidden under the u-phase
    ocur = L.oprew_tiles[(ts // UPRE_WIN) % 2]
    nc.vector.tensor_tensor(L.gpo[0:100, 0:256], L.gpo[0:100, 0:256],
                            ocur[0:100, sw2, :], mybir.AluOpType.add)
    d = L.tmp_pool.tile([128, 256], fp32, tag="d", name="d")
    nc.vector.scalar_tensor_tensor(
        d[0:100, :], L.gpo[0:100, 0:256], 0.0, L.prev_h[0:100, :],
        mybir.AluOpType.max, mybir.AluOpType.subtract)
    # ---- u-phase
    for k in range(8):
        lhsT = (ht[:, k, :] if ht.ndim == 3 else ht[:, 4 * k:4 * k + 4])
        for j in range(4):
            nc.tensor.matmul(
                L.gpu[32 * j:32 * j + 4, 0:256],
                lhsT,
                L.w_t[k][:, 256 * j:256 * j + 256],
                start=(k == 0), stop=(k == 7),
                tile_position=(0, 32 * j),
                skip_group_check=True,
            )
    # ---- tail: u' = sigmoid(-(gu+pu));  t = u'*d;  hn = h + t
    ucur = L.uprew_tiles[(ts // UPRE_WIN) % 2]
    nc.vector.tensor_tensor(L.gpu[0:100, 0:256], L.gpu[0:100, 0:256],
                            ucur[0:100, sw2, :], mybir.AluOpType.add)
    up = L.tmp_pool.tile([128, 256], fp32, tag="up", name="up")
    nc.scalar.activation(up[0:100, :], L.gpu[0:100, 0:256],
                         mybir.ActivationFunctionType.Sigmoid, scale=-1.0)
    nc.vector.tensor_tensor(d[0:100, :], d[0:100, :], up[0:100, :],
                            mybir.AluOpType.mult)
    nc.vector.tensor_tensor(hn[0:100, :], L.prev_h[0:100, :],
                            d[0:100, :], mybir.AluOpType.add)
    L.hn = hn
    L.prev_h = hn


def _emit_rebuild(nc, L, ts):
    """PE: transpose hn into tp; then I4 for step ts+1; DVE: copy tp -> ht."""
    bf16 = mybir.dt.bfloat16
    layer = L.layer
    hn = L.hn
    tp = L.tp

    for k in range(8):
        j, kk = k // 2, k % 2
        nc.tensor.matmul(
            tp[:, 4 * k:4 * k + 4],
            hn[32 * j:32 * j + 4, 128 * kk:128 * kk + 128],
            L.id_tr[32 * j:32 * j + 4, 0:4],
            tile_position=(32 * j, 0),
            skip_group_check=True,
        )
    _emit_gemm_step(nc, L.gemmL)  # PE filler while the copy below lands
    # the ht copies run on the near-idle ACT engine: on DVE they queue
    # behind the other layer's elementwise tail (~650ns extra PE wait)
    if layer == 0:
        sw = ts % P1_WIN
        nc.scalar.activation(
            L.o0t_cur[:, :, 4 * sw:4 * sw + 4],
            tp[:, 0:32].rearrange("p (k b) -> p k b", b=4),
            mybir.ActivationFunctionType.Copy)
        L.ht_src = L.o0t_cur[:, :, 4 * sw:4 * sw + 4]
        if sw == P1_WIN - 1:
            # hand this window's o0t to the spread GEMM (one matmul per
            # rebuild slot over the following window's steps)
            L.gemm_src = L.o0t_cur
            L.gemm_w = ts // P1_WIN
            L.gemm_i = 0
    else:
        ht = L.ht_pool.tile([128, 32], bf16, tag="ht", name="ht")
        nc.scalar.activation(ht[:, :], tp[:, 0:32],
                             mybir.ActivationFunctionType.Copy)
        L.ht_src = ht
        s_st = ts % OUT_WIN
        if s_st == OUT_WIN - 1:
            w0 = ts - (OUT_WIN - 1)
            for j in range(4):
                nc.sync.dma_start(
                    L.out_dram[:, w0:w0 + OUT_WIN, 256 * j:256 * j + 256],
                    L.stage_cur[32 * j:32 * j + 4, :, :],
                )


def _emit_gemm_step(nc, L):
    """Emit one matmul of layer-0's pending layer-1 input GEMM.

    The GEMM is cut into 64 N=256 matmuls so each of the two per-step
    rebuild slots gets one as PE filler over the ht-copy latency."""
    if L is None or L.gemm_src is None:
        return
    fp32 = mybir.dt.float32
    bf16 = mybir.dt.bfloat16
    i = L.gemm_i
    n, k = i // 8, i % 8      # n: 8 column slices of 256, k: 8 K chunks
    if k == 0:
        # full-bank tile (half used) so alternating pp tiles never share a
        # bank: PE-write(pp_n) + DVE-read(pp_{n-1}) same-bank is fatal
        L.gemm_pp = L.pps_pool.tile([128, 512], fp32, tag="pps", name="pps")
        if n == 0:
            L.gemm_psb = L.psb_pool.tile([128, G], bf16, tag="psb",
                                         name="psb")
    nc.tensor.matmul(
        L.gemm_pp[:, 0:256],
        L.gemm_src[:, k, :],
        L.wih1_t[k][:, 256 * n:256 * n + 256],
        start=(k == 0), stop=(k == 7),
        skip_group_check=True,
    )
    if k == 7:
        nc.vector.tensor_tensor(
            L.gemm_psb[:, 256 * n:256 * n + 256], L.gemm_pp[:, 0:256],
            L.b1f_t[:, 256 * n:256 * n + 256], mybir.AluOpType.add)
    L.gemm_i += 1
    if L.gemm_i == 64:
        nc.sync.dma_start(L.pre1_d[L.gemm_w, :, :], L.gemm_psb[:, :])
        L.gemm_src = None


def _build():
    fp32 = mybir.dt.float32
    bf16 = mybir.dt.bfloat16
    nc = bacc.Bacc("TRN2", target_bir_lowering=False, debug=False,
                   num_devices=NCORES)

    xt = nc.dram_tensor("xt", [DX, BL * T], bf16, kind="ExternalInput")
    w0p = nc.dram_tensor("w0p", [H, G], bf16, kind="ExternalInput")
    w1p = nc.dram_tensor("w1p", [H, G], bf16, kind="ExternalInput")
    wih0p = nc.dram_tensor("wih0p", [DX, G], bf16, kind="ExternalInput")
    wih1p = nc.dram_tensor("wih1p", [H, G], bf16, kind="ExternalInput")
    b0f = nc.dram_tensor("b0f", [128, G], fp32, kind="ExternalInput")
    b1f = nc.dram_tensor("b1f", [128, G], fp32, kind="ExternalInput")
    h0t = nc.dram_tensor("h0t", [128, 32], bf16, kind="ExternalInput")
    h1t = nc.dram_tensor("h1t", [128, 32], bf16, kind="ExternalInput")
    h0s = nc.dram_tensor("h0s", [128, 256], fp32, kind="ExternalInput")
    h1s = nc.dram_tensor("h1s", [128, 256], fp32, kind="ExternalInput")
    idt = nc.dram_tensor("idt", [128, 4], fp32, kind="ExternalInput")
    out = nc.dram_tensor("out", [BL, T, H], fp32, kind="ExternalOutput")

    pre0_d = nc.dram_tensor("pre0_d", [BL, T, G], bf16, kind="Internal")
    pre1_d = nc.dram_tensor("pre1_d", [T // P1_WIN, 128, G], bf16,
                            kind="Internal")

    with TileContext(nc) as tc:
        with tc.tile_pool(name="wconst", bufs=1) as wconst:
            w0_t = [wconst.tile([128, G], bf16, tag=f"w0_{k}", name=f"w0_{k}")
                    for k in range(8)]
            w1_t = [wconst.tile([128, G], bf16, tag=f"w1_{k}", name=f"w1_{k}")
                    for k in range(8)]
            wih1_t = [wconst.tile([128, G], bf16, tag=f"wih1_{k}",
                                  name=f"wih1_{k}") for k in range(8)]
            b1f_t = wconst.tile([128, G], fp32, tag="b1f", name="b1f")
            h0t_t = wconst.tile([128, 32], bf16, tag="h0t", name="h0t")
            h1t_t = wconst.tile([128, 32], bf16, tag="h1t", name="h1t")
            h0s_t = wconst.tile([128, 256], fp32, tag="h0s", name="h0s")
            h1s_t = wconst.tile([128, 256], fp32, tag="h1s", name="h1s")
            idt_t = wconst.tile([128, 4], fp32, tag="idt", name="idt")

            # ---- P1: layer-0 input GEMM -> pre0_d (bf16, natural order)
            with (
                tc.tile_pool(name="p1x", bufs=1) as p1x,
                tc.tile_pool(name="p1ps", bufs=2, space="PSUM") as p1ps,
                tc.tile_pool(name="p1o", bufs=2) as p1o,
            ):
                b0f_t = p1x.tile([128, G], fp32, tag="b0f", name="b0f")
                xt_t = [p1x.tile([128, BL * T], bf16, tag=f"xt{k}",
                                 name=f"xtt{k}") for k in range(4)]
                wih0_t = [p1x.tile([128, G], bf16, tag=f"wih0_{k}",
                                   name=f"wih0_{k}") for k in range(4)]
                # P1's own inputs first: the first GEMM matmul only waits on
                # these; the recurrence weights queue behind on SP
                for k in range(4):
                    nc.sync.dma_start(xt_t[k][:, :],
                                      xt[128 * k:128 * k + 128, :])
                    nc.sync.dma_start(wih0_t[k][:, :],
                                      wih0p[128 * k:128 * k + 128, :])
                nc.sync.dma_start(b0f_t[:, :], b0f[:, :])
                for k in range(8):
                    nc.sync.dma_start(w0_t[k][:, :],
                                      w0p[128 * k:128 * k + 128, :])
                    nc.sync.dma_start(w1_t[k][:, :],
                                      w1p[128 * k:128 * k + 128, :])
                    nc.sync.dma_start(wih1_t[k][:, :],
                                      wih1p[128 * k:128 * k + 128, :])
                nc.sync.dma_start(b1f_t[:, :], b1f[:, :])
                for dst, src in ((h0t_t, h0t), (h1t_t, h1t), (h0s_t, h0s),
                                 (h1s_t, h1s), (idt_t, idt)):
                    nc.sync.dma_start(dst[:, :], src[:, :])
                for m in range(BL * T // 128):
                    po = p1o.tile([128, G], bf16, tag="po", name="po")
                    for n in range(4):
                        pp = p1ps.tile([128, 512], fp32, tag="pp", name="pp")
                        for k in range(4):
                            nc.tensor.matmul(
                                pp[:, :],
                                xt_t[k][:, 128 * m:128 * m + 128],
                                wih0_t[k][:, 512 * n:512 * n + 512],
                                start=(k == 0), stop=(k == 3),
                            )
                        nc.vector.tensor_tensor(
                            po[:, 512 * n:512 * n + 512], pp[:, :],
                            b0f_t[:, 512 * n:512 * n + 512],
                            mybir.AluOpType.add)
                    bb = m // (T // 128)
                    t0 = 128 * (m % (T // 128))
                    nc.sync.dma_start(pre0_d[bb, t0:t0 + 128, :], po[:, :])

            tc.strict_bb_all_engine_barrier()
            with contextlib.ExitStack() as stack:
                L0 = _make_layer(nc, tc, stack, 0, w0_t, pre0_d, h0t_t,
                                 h0s_t, idt_t, wih1_t=wih1_t, b1f_t=b1f_t,
                                 pre1_d=pre1_d)
                L1 = _make_layer(nc, tc, stack, 1, w1_t, pre1_d, h1t_t,
                                 h1s_t, idt_t, out_dram=out)
                for L in (L0, L1):
                    L.id_tr = idt_t     # fp32 identity for the transposes
                    L.gemmL = L0        # both rebuild slots drain L0's GEMM
                _emit_sprew_dma(nc, L0, 0, 0)
                _emit_sprew_dma(nc, L0, 0, 1)
                for tt in range(T + LAG + 1):
                    if tt < T:
                        _emit_gates(nc, L0, tt)
                    if tt == LAG - 2:
                        # L1's first pre windows: emitted only after L0's
                        # window-0 GEMM wrote pre1_d[0] (program order).
                        _emit_sprew_dma(nc, L1, 0, 0)
                        _emit_sprew_dma(nc, L1, 0, 1)
                    s1 = tt - LAG - 1
                    if 0 <= s1 < T:
                        _emit_rebuild(nc, L1, s1)
                    s1b = tt - LAG
                    if 0 <= s1b < T:
                        _emit_gates(nc, L1, s1b)
                    if tt < T:
                        _emit_rebuild(nc, L0, tt)
                        if tt <= LAG:
                            # L1's rebuild slot doesn't exist yet: pull an
                            # extra GEMM matmul so window 0 drains in time
                            _emit_gemm_step(nc, L0)
                    else:
                        # drain the last window's spread GEMM
                        _emit_gemm_step(nc, L0)
                        _emit_gemm_step(nc, L0)

    nc.compile()
    return nc


def _prep_core(inputs, c, shared):
    x = inputs["x"][BL * c: BL * c + BL, :T]          # [4, T, DX]
    xt = np.ascontiguousarray(
        x.transpose(2, 0, 1).reshape(DX, BL * T)).astype(BF16)

    def hscat(hv):                                    # [4, H] -> [128, 256]
        o = np.zeros((128, 256), F32)
        for j in range(4):
            o[32 * j: 32 * j + 4, :] = hv[:, 256 * j: 256 * j + 256]
        return o

    def htr(hv):                                      # [4, H] -> [128, 32]
        o = np.zeros((128, 32), F32)
        for k in range(8):
            o[:, 4 * k: 4 * k + 4] = hv[:, 128 * k: 128 * k + 128].T
        return o

    h0 = inputs["hx"][0, BL * c: BL * c + BL]
    h1 = inputs["hx"][1, BL * c: BL * c + BL]
    return {
        "xt": xt,
        "h0t": htr(h0).astype(BF16), "h1t": htr(h1).astype(BF16),
        "h0s": hscat(h0), "h1s": hscat(h1),
        **shared,
    }


def get_nc():
    nc = _CACHE.get("nc")
    if nc is None:
        nc = _build()
        _CACHE["nc"] = nc
    return nc


def make_in_maps(inputs):
    inputs = {k: np.asarray(v) for k, v in inputs.items()}
    idt = np.zeros((128, 4), F32)
    for j in range(4):
        for b in range(4):
            idt[32 * j + b, b] = 1.0
    shared = {
        "w0p": np.ascontiguousarray(inputs["w_hh_l0"].T).astype(BF16),
        "w1p": np.ascontiguousarray(inputs["w_hh_l1"].T).astype(BF16),
        "wih0p": np.ascontiguousarray(inputs["w_ih_l0"].T).astype(BF16),
        "wih1p": np.ascontiguousarray(inputs["w_ih_l1"].T).astype(BF16),
        "b0f": np.broadcast_to(
            (inputs["b_ih_l0"] + inputs["b_hh_l0"]),
            (128, G)).astype(F32).copy(),
        "b1f": np.broadcast_to(
            (inputs["b_ih_l1"] + inputs["b_hh_l1"]),
            (128, G)).astype(F32).copy(),
        "idt": idt,
    }
    return [_prep_core(inputs, c, shared) for c in range(NCORES)]


def kernel(**inputs) -> np.ndarray:
    nc = get_nc()
    in_maps = make_in_maps(inputs)
    try:
        res = run_bass_kernel_spmd(nc, in_maps, core_ids=list(range(NCORES)))
    except Exception:
        # a previously wedged device often recovers on the next attempt
        import time
        time.sleep(2.0)
        res = run_bass_kernel_spmd(nc, in_maps, core_ids=list(range(NCORES)))
    out = np.concatenate([res.results[c]["out"] for c in range(NCORES)],
                         axis=0)
    return np.asarray(out, np.float32)
